# revision 3
# baseline (speedup 1.0000x reference)
"""AdaptiveMemoryBank retrieval-KNN kernel for 8 TRN2 NeuronCores (Bass).

Self-contained: builds one SPMD Bass graph (identical on all 8 cores),
shards inputs, runs via concourse.bass_utils.run_bass_kernel_spmd, and
reassembles full outputs.

Algorithm (see _build_graph):
  - memory_bank rows sharded 8-way; queries replicated for the similarity
    phase; the final merge/gather is sharded by 512-query slices, routed
    with an AllToAll so the SPMD graph needs no core-dependent addressing.
  - Projections (q@Wk.T+bk, bank@Wk.T) in native fp32 matmul; bias is
    applied only on the query side (the memory-side bias shifts every
    similarity row by a constant and cannot change the top-k ranking).
  - Similarity uses a 3-pass compensated fp32r (TF32) matmul:
    x = r + e (tf32 split); sim = qr.mr + qr.me + qe.mr accumulated in
    fp32 PSUM. This reproduces fp32 accuracy at 3 cycles/row instead of
    native fp32's 4.
  - Top-k: per 128-wide similarity block, the VectorE max/max_index
    instructions extract the top-8 values+positions; block candidates are
    merged with 12-pass max/max_index/match_replace top-32 stages, with
    per-partition index selection done by a double-local_scatter rank
    trick on GPSIMD. Final memory rows are fetched with one-row-per-
    partition indirect DMAs from a replicated copy of the bank.
Host side: concatenates per-core outputs and applies the usage update
(np.add.at, matching jax scatter-add of 0.1 per hit).
"""
import sys
sys.path.insert(0, '/opt/trn_rl_repo')
import numpy as np
from concourse import bass, mybir, bacc
from concourse.tile import TileContext
from concourse.masks import make_identity

F32 = mybir.dt.float32
F32R = mybir.dt.float32r
BF = mybir.dt.bfloat16
U16 = mybir.dt.uint16
I16 = mybir.dt.int16
I32 = mybir.dt.int32
AT = mybir.AluOpType


def _topk32(nc, work, vals_out, pos_out, n_rounds=4):
    """In-place top-(8*n) of `work` (destroyed): desc values + u16 positions."""
    for r in range(n_rounds):
        v8 = vals_out[:, r * 8:(r + 1) * 8]
        p8 = pos_out[:, r * 8:(r + 1) * 8]
        nc.vector.max(out=v8, in_=work[:])
        nc.vector.max_index(out=p8, in_max=v8, in_values=work[:])
        if r < n_rounds - 1:
            nc.vector.match_replace(out=work[:], in_to_replace=v8,
                                    in_values=work[:], imm_value=-1e30)


def _map_pos(nc, sb, pos_u16, idx_arr, ranks32, out_idx, W):
    """out_idx[p, r] = idx_arr[p, pos_u16[p, r]] via double local_scatter."""
    pos16 = sb.tile([128, 32], I16, tag="mp_pos16")
    nc.vector.tensor_copy(pos16[:], pos_u16[:])
    rk = sb.tile([128, W], U16, tag="mp_rk")
    nc.gpsimd.local_scatter(out_ap=rk[:], data_ap=ranks32[:], idxs_ap=pos16[:],
                            channels=128, num_elems=W, num_idxs=32)
    rkm1 = sb.tile([128, W], I16, tag="mp_rkm1")
    nc.vector.tensor_scalar(rkm1[:], rk[:], 1.0, None, op0=AT.subtract)
    nc.gpsimd.local_scatter(out_ap=out_idx[:], data_ap=idx_arr[:],
                            idxs_ap=rkm1[:], channels=128, num_elems=32,
                            num_idxs=W)


def _build_graph(B, M, D, CORES=8, NQ=8, K=32, single_core=False,
                skip_topk=False, sim_passes=3, skip_gather=False,
                stop_after=99):
    MS = M // CORES            # bank rows per core
    MQ = MS // NQ              # rows per m-slice
    NCH = max(1, MQ // 512)    # psum chunks per m-slice
    CH = MQ // NCH             # psum chunk width (<=512)
    NB = MQ // 128             # 128-wide blocks per m-slice
    CW = NB * 8                # block candidates per (slice, tile)
    DT = D // 128
    QT = B // 128
    QS = B // CORES
    QTS = QS // 128
    QCH = B // 512
    W2 = NQ * CW               # local merge width (512 for full size)
    W3 = CORES * K             # global merge width

    nc = bacc.Bacc("TRN2", target_bir_lowering=False, debug=False,
                   num_devices=1 if single_core else CORES)
    query = nc.declare_dram_parameter("query", [B, D], F32, isOutput=False)
    Wk = nc.declare_dram_parameter("Wk", [D, D], F32, isOutput=False)
    bkp = nc.declare_dram_parameter("bk", [1, D], F32, isOutput=False)
    bank_shard = nc.declare_dram_parameter("bank_shard", [MS, D], F32,
                                           isOutput=False)
    bank_full = nc.declare_dram_parameter("bank_full", [M, D], F32,
                                          isOutput=False)
    retr = nc.declare_dram_parameter("retr", [QS, K, D], F32, isOutput=True)
    fidx = nc.declare_dram_parameter("fidx", [QS, K], U16, isOutput=True)

    with TileContext(nc) as tc:
        with tc.tile_pool(name="dram", bufs=1, space="DRAM") as dram, \
             tc.tile_pool(name="const", bufs=1) as cst, \
             tc.tile_pool(name="psum", bufs=4, space="PSUM") as ps:
            # [dout_t][p][dint][dj]: per-partition-contiguous slices
            wkt_d = dram.tile([DT, 128, DT, 128], F32, tag="wkt_d")
            qkr_d = dram.tile([QT, 128, DT, 128], F32, tag="qkr_d")
            qke_d = dram.tile([QT, 128, DT, 128], F32, tag="qke_d")
            lv_d = dram.tile([QT, 128, NQ, CW], F32, tag="lv_d")
            li_d = dram.tile([QT, 128, NQ, CW], U16, tag="li_d")
            av_in = dram.tile([B, K], F32, tag="av_in")
            ai_in = dram.tile([B, K], U16, tag="ai_in")
            av_out = dram.tile([B, K], F32, tag="av_out")
            ai_out = dram.tile([B, K], U16, tag="ai_out")

            ident = cst.tile([128, 128], F32, tag="ident")
            make_identity(nc, ident[:])
            bk_sb = cst.tile([1, D], F32, tag="bk_sb")
            nc.sync.dma_start(bk_sb[:], bkp[:, :])
            ones = cst.tile([1, 512], F32, tag="ones")
            nc.vector.memset(ones[:], 1.0)
            ranks32 = cst.tile([128, 32], U16, tag="ranks32")
            nc.gpsimd.iota(ranks32[:], pattern=[[1, 32]], base=1,
                           channel_multiplier=0)
            # block position offsets for P3: e*MQ + b*128 over [NQ, NB, 8]
            boff = cst.tile([128, NQ, NB, 8], U16, tag="boff")
            nc.gpsimd.iota(boff[:], pattern=[[MQ, NQ], [128, NB], [0, 8]],
                           base=0, channel_multiplier=0)
            # shard offsets for global merge
            shoff = cst.tile([128, CORES, K], U16, tag="shoff")
            nc.gpsimd.iota(shoff[:], pattern=[[MS, CORES], [0, K]], base=0,
                           channel_multiplier=0)
            # ---------------- P0: WkT ----------------
            with tc.tile_pool(name="p0", bufs=2) as p0:
                wrows = p0.tile([128, DT, D], F32, tag="wrows")
                nc.sync.dma_start(
                    wrows[:], Wk[:, :].rearrange("(dt p) d -> p dt d", p=128))
                for dout_t in range(DT):
                    wkt_sb = p0.tile([128, DT, 128], F32, tag="wkt_sb")
                    for dint in range(DT):
                        pt = ps.tile([128, 128], F32, tag="pt_t")
                        nc.tensor.transpose(
                            out=pt[:],
                            in_=wrows[:, dout_t, dint * 128:(dint + 1) * 128],
                            identity=ident[:])
                        nc.scalar.copy(out=wkt_sb[:, dint, :], in_=pt[:])
                    nc.sync.dma_start(wkt_d[dout_t, :, :, :], wkt_sb[:])

            # ---------------- P1: qk projection ----------------
            if stop_after >= 1:
                with tc.tile_pool(name="p1", bufs=2) as p1, \
                     tc.tile_pool(name="p1w", bufs=1) as p1w:
                    wkt_all = p1w.tile([128, DT, DT, 128], F32, tag="wkt_all")
                    nc.sync.dma_start(
                        wkt_all[:],
                        wkt_d[:, :, :, :].rearrange("do p di q -> p do di q"))
                    for ch in range(QCH):
                        qrows = p1.tile([128, 4, D], F32, tag="qrows")
                        nc.sync.dma_start(
                            qrows[:],
                            query[ch * 512:(ch + 1) * 512, :].rearrange(
                                "(qb p) d -> p qb d", p=128))
                        qt_ch = p1.tile([128, DT, 512], F32, tag="qt_ch")
                        for qb in range(4):
                            for dint in range(DT):
                                pt = ps.tile([128, 128], F32, tag="pt_t")
                                nc.tensor.transpose(
                                    out=pt[:],
                                    in_=qrows[:, qb,
                                              dint * 128:(dint + 1) * 128],
                                    identity=ident[:])
                                nc.scalar.copy(
                                    out=qt_ch[:, dint,
                                              qb * 128:(qb + 1) * 128],
                                    in_=pt[:])
                        for dout_t in range(DT):
                            pj = ps.tile([128, 512], F32, tag="pj")
                            for dint in range(DT):
                                nc.tensor.matmul(
                                    pj[:], wkt_all[:, dout_t, dint, :],
                                    qt_ch[:, dint, :],
                                    start=(dint == 0), stop=False)
                            nc.tensor.matmul(
                                pj[:],
                                bk_sb[:1, dout_t * 128:(dout_t + 1) * 128],
                                ones[:1, :], start=False, stop=True)
                            qk_f = p1.tile([128, 512], F32, tag="qk_f")
                            nc.scalar.copy(out=qk_f[:], in_=pj[:])
                            qk_r = p1.tile([128, 512], F32R, tag="qk_r")
                            qk_e = p1.tile([128, 512], F32R, tag="qk_e")
                            tmp = p1.tile([128, 512], F32, tag="qk_tmp")
                            nc.vector.tensor_copy(qk_r[:], qk_f[:])
                            nc.vector.tensor_tensor(out=tmp[:], in0=qk_f[:],
                                                    in1=qk_r[:].bitcast(F32),
                                                    op=AT.subtract)
                            nc.vector.tensor_copy(qk_e[:], tmp[:])
                            nc.sync.dma_start(
                                qkr_d[ch * 4:(ch + 1) * 4, :, dout_t, :]
                                .rearrange("tb p q -> p tb q"),
                                qk_r[:].bitcast(F32).rearrange(
                                    "p (tb q) -> p tb q", tb=4))
                            nc.sync.dma_start(
                                qke_d[ch * 4:(ch + 1) * 4, :, dout_t, :]
                                .rearrange("tb p q -> p tb q"),
                                qk_e[:].bitcast(F32).rearrange(
                                    "p (tb q) -> p tb q", tb=4))

            # ---------------- P2: m-slices ----------------
            if stop_after >= 2:
                with tc.tile_pool(name="mkp", bufs=2) as mkp, \
                     tc.tile_pool(name="p2a", bufs=1) as p2a, \
                     tc.tile_pool(name="p2b", bufs=2) as p2b:
                    for e in range(NQ):
                        mkr = mkp.tile([128, DT, MQ], F32R, tag="mkr")
                        mke = mkp.tile([128, DT, MQ], F32R, tag="mke")
                        ACH = min(256, MQ)
                        for mc in range(MQ // ACH):
                            m0 = e * MQ + mc * ACH
                            brows = p2a.tile([128, ACH // 128, D], F32,
                                             tag="brows")
                            nc.sync.dma_start(
                                brows[:],
                                bank_shard[m0:m0 + ACH, :].rearrange(
                                    "(mb p) d -> p mb d", p=128))
                            bT = p2a.tile([128, DT, ACH], F32, tag="bT")
                            for mb in range(ACH // 128):
                                for dint in range(DT):
                                    pt = ps.tile([128, 128], F32, tag="pt_t")
                                    nc.tensor.transpose(
                                        out=pt[:],
                                        in_=brows[:, mb,
                                                  dint * 128:(dint + 1) * 128],
                                        identity=ident[:])
                                    nc.scalar.copy(
                                        out=bT[:, dint,
                                               mb * 128:(mb + 1) * 128],
                                        in_=pt[:])
                            for dout_t in range(DT):
                                wslice = p2a.tile([128, DT, 128], F32,
                                                  tag="wslice")
                                nc.sync.dma_start(wslice[:],
                                                  wkt_d[dout_t, :, :, :])
                                pj = ps.tile([128, 512], F32, tag="pj")
                                for dint in range(DT):
                                    nc.tensor.matmul(
                                        pj[:, :ACH], wslice[:, dint, :],
                                        bT[:, dint, :],
                                        start=(dint == 0),
                                        stop=(dint == DT - 1))
                                mk_f = p2a.tile([128, ACH], F32, tag="mk_f")
                                nc.scalar.copy(out=mk_f[:], in_=pj[:, :ACH])
                                tmp2 = p2a.tile([128, ACH], F32, tag="mk_tmp")
                                sl = slice(mc * ACH, (mc + 1) * ACH)
                                nc.vector.tensor_copy(mkr[:, dout_t, sl],
                                                      mk_f[:])
                                nc.vector.tensor_tensor(
                                    out=tmp2[:], in0=mk_f[:],
                                    in1=mkr[:, dout_t, sl].bitcast(F32),
                                    op=AT.subtract)
                                nc.vector.tensor_copy(mke[:, dout_t, sl],
                                                      tmp2[:])
                        for t in range(QT):
                            qkr_t = p2b.tile([128, DT, 128], F32R,
                                             tag="qkr_t")
                            qke_t = p2b.tile([128, DT, 128], F32R,
                                             tag="qke_t")
                            nc.sync.dma_start(qkr_t[:].bitcast(F32),
                                              qkr_d[t, :, :, :])
                            nc.sync.dma_start(qke_t[:].bitcast(F32),
                                              qke_d[t, :, :, :])
                            sim = p2b.tile([128, MQ], F32, tag="sim")
                            passes = (((qkr_t, mkr), (qkr_t, mke),
                                       (qke_t, mkr))[:sim_passes])
                            for mc in range(NCH):
                                pj = ps.tile([128, CH], F32, tag="pj")
                                nmm = DT * len(passes)
                                i = 0
                                for dint in range(DT):
                                    sl = slice(mc * CH, (mc + 1) * CH)
                                    for (lh, rh) in passes:
                                        nc.tensor.matmul(
                                            pj[:, :CH], lh[:, dint, :],
                                            rh[:, dint, sl],
                                            start=(i == 0),
                                            stop=(i == nmm - 1))
                                        i += 1
                                nc.scalar.copy(
                                    out=sim[:, mc * CH:(mc + 1) * CH],
                                    in_=pj[:, :CH])
                            bv = p2b.tile([128, NB, 8], F32, tag="bv")
                            bp = p2b.tile([128, NB, 8], U16, tag="bp")
                            if skip_topk:
                                nc.vector.tensor_copy(bv[:], sim[:, :NB * 8])
                                nc.vector.tensor_copy(bp[:], bv[:])
                            else:
                                for b in range(NB):
                                    blk = sim[:, b * 128:(b + 1) * 128]
                                    nc.vector.max(out=bv[:, b, :], in_=blk)
                                    nc.vector.max_index(
                                        out=bp[:, b, :], in_max=bv[:, b, :],
                                        in_values=blk)
                            nc.sync.dma_start(lv_d[t, :, e, :], bv[:])
                            nc.sync.dma_start(li_d[t, :, e, :], bp[:])

            # ---------------- P3: local merge + A2A prep ----------------
            if stop_after >= 3:
                with tc.tile_pool(name="p3", bufs=2) as p3:
                    for t in range(QT):
                        cv = p3.tile([128, NQ, CW], F32, tag="cv3")
                        ci = p3.tile([128, NQ, CW], U16, tag="ci3")
                        nc.sync.dma_start(cv[:], lv_d[t, :, :, :])
                        nc.sync.dma_start(ci[:], li_d[t, :, :, :])
                        cig = p3.tile([128, NQ, CW], U16, tag="cig3")
                        nc.vector.tensor_tensor(
                            out=cig[:], in0=ci[:],
                            in1=boff[:].rearrange("p e nb k -> p e (nb k)"),
                            op=AT.add)
                        work = cv[:].rearrange("p e k -> p (e k)")
                        flv = p3.tile([128, K], F32, tag="flv")
                        fpos = p3.tile([128, K], U16, tag="fpos")
                        _topk32(nc, work, flv, fpos)
                        fli = p3.tile([128, K], U16, tag="fli")
                        _map_pos(nc, p3, fpos,
                                cig[:].rearrange("p e k -> p (e k)"),
                                ranks32, fli[:], W2)
                        nc.sync.dma_start(av_in[t * 128:(t + 1) * 128, :],
                                          flv[:])
                        nc.sync.dma_start(ai_in[t * 128:(t + 1) * 128, :],
                                          fli[:])

            # ---------------- P4: AllToAll ----------------
            if stop_after >= 4:
                if single_core:
                    nc.gpsimd.dma_start(av_out[:, :], av_in[:, :])
                    nc.gpsimd.dma_start(ai_out[:, :], ai_in[:, :])
                else:
                    nc.gpsimd.collective_compute(
                        "AllToAll", AT.bypass,
                        replica_groups=[list(range(CORES))],
                        ins=[av_in.opt()], outs=[av_out.opt()])
                    nc.gpsimd.collective_compute(
                        "AllToAll", AT.bypass,
                        replica_groups=[list(range(CORES))],
                        ins=[ai_in.opt()], outs=[ai_out.opt()])

            # ---------------- P5: global merge + gather ----------------
            if stop_after >= 5:
                with tc.tile_pool(name="p5", bufs=2) as p5:
                    for st in range(QTS):
                        cw = p5.tile([128, CORES, K], F32, tag="cw5")
                        ci = p5.tile([128, CORES, K], U16, tag="ci5")
                        nc.sync.dma_start(
                            cw[:], av_out[:, :].rearrange(
                                "(s q) k -> s q k",
                                s=CORES)[:, st * 128:(st + 1) * 128, :]
                            .rearrange("s p k -> p s k"))
                        nc.sync.dma_start(
                            ci[:], ai_out[:, :].rearrange(
                                "(s q) k -> s q k",
                                s=CORES)[:, st * 128:(st + 1) * 128, :]
                            .rearrange("s p k -> p s k"))
                        cig = p5.tile([128, CORES, K], U16, tag="cig")
                        nc.vector.tensor_tensor(out=cig[:], in0=ci[:],
                                                in1=shoff[:], op=AT.add)
                        gv = p5.tile([128, K], F32, tag="gv")
                        gpos = p5.tile([128, K], U16, tag="gpos")
                        _topk32(nc, cw[:].rearrange("p s k -> p (s k)"),
                               gv, gpos)
                        gidx = p5.tile([128, K], U16, tag="gidx")
                        _map_pos(nc, p5, gpos,
                                cig[:].rearrange("p s k -> p (s k)"),
                                ranks32, gidx[:], W3)
                        nc.sync.dma_start(fidx[st * 128:(st + 1) * 128, :],
                                          gidx[:])
                        gidx32 = p5.tile([128, K], I32, tag="gidx32")
                        nc.vector.tensor_copy(gidx32[:], gidx[:])
                        if skip_gather:
                            continue
                        for j in range(K):
                            row = p5.tile([128, D], F32, tag="row")
                            nc.gpsimd.indirect_dma_start(
                                out=row[:], out_offset=None,
                                in_=bank_full[:, :],
                                in_offset=bass.IndirectOffsetOnAxis(
                                    ap=gidx32[:, j:j + 1], axis=0))
                            nc.sync.dma_start(
                                retr[st * 128:(st + 1) * 128, j, :], row[:])

    nc.compile()
    return nc




_GRAPH_CACHE = {}


def kernel(**inputs):
    from concourse import bass_utils
    query = np.ascontiguousarray(np.asarray(inputs["query"]), dtype=np.float32)
    Wk = np.ascontiguousarray(np.asarray(inputs["Wk"]), dtype=np.float32)
    bk = np.ascontiguousarray(np.asarray(inputs["bk"]), dtype=np.float32)
    bank = np.ascontiguousarray(np.asarray(inputs["memory_bank"]),
                                dtype=np.float32)
    usage0 = np.asarray(inputs["memory_usage"])
    k = int(np.asarray(inputs["k"]))
    B, D = query.shape
    M = bank.shape[0]
    CORES = 8
    assert k == 32 and B == 4096 and M == 65536 and D == 1024

    key = (B, M, D)
    if key not in _GRAPH_CACHE:
        _GRAPH_CACHE[key] = _build_graph(B, M, D, CORES=CORES, NQ=8, K=k)
    nc = _GRAPH_CACHE[key]

    MS = M // CORES
    in_maps = []
    for c in range(CORES):
        in_maps.append({
            "query": query,
            "Wk": Wk,
            "bk": bk.reshape(1, D),
            "bank_shard": np.ascontiguousarray(bank[c * MS:(c + 1) * MS]),
            "bank_full": bank,
        })
    res = bass_utils.run_bass_kernel_spmd(nc, in_maps,
                                          core_ids=list(range(CORES)))
    QS = B // CORES
    retr = np.concatenate(
        [np.asarray(r["retr"]).reshape(QS, k, D) for r in res.results], axis=0)
    idx = np.concatenate([np.asarray(r["fidx"]) for r in res.results],
                         axis=0).astype(np.int64)
    usage = np.array(usage0, dtype=np.float32, copy=True)
    np.add.at(usage, idx.ravel(), np.float32(0.1))
    return retr, usage


# revision 4
# speedup vs baseline: 1.8640x; 1.8640x over previous
"""AdaptiveMemoryBank retrieval-KNN kernel for 8 TRN2 NeuronCores (Bass).

Self-contained: builds one SPMD Bass graph (identical on all 8 cores),
shards inputs, runs via concourse.bass_utils.run_bass_kernel_spmd, and
reassembles full outputs.

Algorithm (see _build_graph):
  - memory_bank rows sharded 8-way; queries replicated for the similarity
    phase; the final merge/gather is sharded by 512-query slices, routed
    with an AllToAll so the SPMD graph needs no core-dependent addressing.
  - Projections (q@Wk.T+bk, bank@Wk.T) in native fp32 matmul; bias is
    applied only on the query side (the memory-side bias shifts every
    similarity row by a constant and cannot change the top-k ranking).
  - Similarity uses a 3-pass compensated fp32r (TF32) matmul:
    x = r + e (tf32 split); sim = qr.mr + qr.me + qe.mr accumulated in
    fp32 PSUM. This reproduces fp32 accuracy at 3 cycles/row instead of
    native fp32's 4. The hi/lo pair is stored interleaved in one DRAM
    tensor so each query tile is a single contiguous DMA.
  - Top-k: per 128-wide similarity block, the VectorE max/max_index
    instructions extract the top-8 values+positions (stored as one fused
    value/position plane pair per m-slice); candidates are merged with
    12-pass max/max_index/match_replace top-32 stages, with per-partition
    index selection done by a double-local_scatter rank trick on GPSIMD.
    Final memory rows are fetched with one-row-per-partition indirect
    DMAs from a replicated copy of the bank.
Host side: concatenates per-core outputs and applies the usage update
(np.add.at, matching jax scatter-add of 0.1 per hit).
"""
import sys
sys.path.insert(0, '/opt/trn_rl_repo')
import numpy as np
from concourse import bass, mybir, bacc
from concourse.tile import TileContext
from concourse.masks import make_identity

F32 = mybir.dt.float32
F32R = mybir.dt.float32r
BF = mybir.dt.bfloat16
U16 = mybir.dt.uint16
I16 = mybir.dt.int16
I32 = mybir.dt.int32
AT = mybir.AluOpType


def _topk32(nc, work, vals_out, pos_out, n_rounds=4):
    """In-place top-(8*n) of `work` (destroyed): desc values + u16 positions."""
    for r in range(n_rounds):
        v8 = vals_out[:, r * 8:(r + 1) * 8]
        p8 = pos_out[:, r * 8:(r + 1) * 8]
        nc.vector.max(out=v8, in_=work[:])
        nc.vector.max_index(out=p8, in_max=v8, in_values=work[:])
        if r < n_rounds - 1:
            nc.vector.match_replace(out=work[:], in_to_replace=v8,
                                    in_values=work[:], imm_value=-1e30)


def _map_pos(nc, sb, pos_u16, idx_arr, ranks32, out_idx, W):
    """out_idx[p, r] = idx_arr[p, pos_u16[p, r]] via double local_scatter."""
    pos16 = sb.tile([128, 32], I16, tag="mp_pos16")
    nc.vector.tensor_copy(pos16[:], pos_u16[:])
    rk = sb.tile([128, W], U16, tag="mp_rk")
    nc.gpsimd.local_scatter(out_ap=rk[:], data_ap=ranks32[:], idxs_ap=pos16[:],
                            channels=128, num_elems=W, num_idxs=32)
    rkm1 = sb.tile([128, W], I16, tag="mp_rkm1")
    nc.vector.tensor_scalar(rkm1[:], rk[:], 1.0, None, op0=AT.subtract)
    nc.gpsimd.local_scatter(out_ap=out_idx[:], data_ap=idx_arr[:],
                            idxs_ap=rkm1[:], channels=128, num_elems=32,
                            num_idxs=W)


def _build_graph(B, M, D, CORES=8, NQ=8, K=32, single_core=False,
                skip_topk=False, sim_passes=3, skip_gather=False,
                stop_after=99):
    MS = M // CORES            # bank rows per core
    MQ = MS // NQ              # rows per m-slice
    NCH = max(1, MQ // 512)    # psum chunks per m-slice
    CH = MQ // NCH             # psum chunk width (<=512)
    NB = MQ // 128             # 128-wide blocks per m-slice
    CW = NB * 8                # block candidates per (slice, tile)
    DT = D // 128
    QT = B // 128
    QS = B // CORES
    QTS = QS // 128
    QCH = B // 512
    W2 = NQ * CW               # local merge width (512 for full size)
    W3 = CORES * K             # global merge width

    nc = bacc.Bacc("TRN2", target_bir_lowering=False, debug=False,
                   num_devices=1 if single_core else CORES)
    query = nc.declare_dram_parameter("query", [B, D], F32, isOutput=False)
    Wk = nc.declare_dram_parameter("Wk", [D, D], F32, isOutput=False)
    bkp = nc.declare_dram_parameter("bk", [1, D], F32, isOutput=False)
    bank_shard = nc.declare_dram_parameter("bank_shard", [MS, D], F32,
                                           isOutput=False)
    bank_full = nc.declare_dram_parameter("bank_full", [M, D], F32,
                                          isOutput=False)
    retr = nc.declare_dram_parameter("retr", [QS, K, D], F32, isOutput=True)
    fidx = nc.declare_dram_parameter("fidx", [QS, K], U16, isOutput=True)

    with TileContext(nc) as tc:
        with tc.tile_pool(name="dram", bufs=1, space="DRAM") as dram, \
             tc.tile_pool(name="const", bufs=1) as cst, \
             tc.tile_pool(name="psum", bufs=4, space="PSUM") as ps:
            # [dout_t][p][dint][dj]: per-partition-contiguous slices
            wkt_d = dram.tile([DT, 128, DT, 128], F32, tag="wkt_d")
            qkp_d = dram.tile([QT, 128, 2, DT, 128], F32, tag="qkp_d")
            lvp_d = dram.tile([QT, 128, NQ, 2, CW], F32, tag="lvp_d")
            av_in = dram.tile([B, K], F32, tag="av_in")
            ai_in = dram.tile([B, K], U16, tag="ai_in")
            av_out = dram.tile([B, K], F32, tag="av_out")
            ai_out = dram.tile([B, K], U16, tag="ai_out")

            ident = cst.tile([128, 128], F32, tag="ident")
            make_identity(nc, ident[:])
            bk_sb = cst.tile([1, D], F32, tag="bk_sb")
            nc.sync.dma_start(bk_sb[:], bkp[:, :])
            ones = cst.tile([1, 512], F32, tag="ones")
            nc.vector.memset(ones[:], 1.0)
            ranks32 = cst.tile([128, 32], U16, tag="ranks32")
            nc.gpsimd.iota(ranks32[:], pattern=[[1, 32]], base=1,
                           channel_multiplier=0)
            # block position offsets for P3: e*MQ + b*128 over [NQ, NB, 8]
            boff = cst.tile([128, NQ, NB, 8], U16, tag="boff")
            nc.gpsimd.iota(boff[:], pattern=[[MQ, NQ], [128, NB], [0, 8]],
                           base=0, channel_multiplier=0)
            # shard offsets for global merge
            shoff = cst.tile([128, CORES, K], U16, tag="shoff")
            nc.gpsimd.iota(shoff[:], pattern=[[MS, CORES], [0, K]], base=0,
                           channel_multiplier=0)
            # ---------------- P0: WkT ----------------
            with tc.tile_pool(name="p0", bufs=2) as p0:
                wrows = p0.tile([128, DT, D], F32, tag="wrows")
                nc.sync.dma_start(
                    wrows[:], Wk[:, :].rearrange("(dt p) d -> p dt d", p=128))
                for dout_t in range(DT):
                    wkt_sb = p0.tile([128, DT, 128], F32, tag="wkt_sb")
                    for dint in range(DT):
                        pt = ps.tile([128, 128], F32, tag="pt_t")
                        nc.tensor.transpose(
                            out=pt[:],
                            in_=wrows[:, dout_t, dint * 128:(dint + 1) * 128],
                            identity=ident[:])
                        nc.scalar.copy(out=wkt_sb[:, dint, :], in_=pt[:])
                    nc.sync.dma_start(wkt_d[dout_t, :, :, :], wkt_sb[:])

            # ---------------- P1: qk projection ----------------
            if stop_after >= 1:
                with tc.tile_pool(name="p1", bufs=2) as p1, \
                     tc.tile_pool(name="p1w", bufs=1) as p1w:
                    wkt_all = p1w.tile([128, DT, DT, 128], F32, tag="wkt_all")
                    nc.sync.dma_start(
                        wkt_all[:],
                        wkt_d[:, :, :, :].rearrange("do p di q -> p do di q"))
                    for ch in range(QCH):
                        qrows = p1.tile([128, 4, D], F32, tag="qrows")
                        nc.sync.dma_start(
                            qrows[:],
                            query[ch * 512:(ch + 1) * 512, :].rearrange(
                                "(qb p) d -> p qb d", p=128))
                        qt_ch = p1.tile([128, DT, 512], F32, tag="qt_ch")
                        for qb in range(4):
                            for dint in range(DT):
                                pt = ps.tile([128, 128], F32, tag="pt_t")
                                nc.tensor.transpose(
                                    out=pt[:],
                                    in_=qrows[:, qb,
                                              dint * 128:(dint + 1) * 128],
                                    identity=ident[:])
                                nc.scalar.copy(
                                    out=qt_ch[:, dint,
                                              qb * 128:(qb + 1) * 128],
                                    in_=pt[:])
                        for dout_t in range(DT):
                            pj = ps.tile([128, 512], F32, tag="pj")
                            for dint in range(DT):
                                nc.tensor.matmul(
                                    pj[:], wkt_all[:, dout_t, dint, :],
                                    qt_ch[:, dint, :],
                                    start=(dint == 0), stop=False)
                            nc.tensor.matmul(
                                pj[:],
                                bk_sb[:1, dout_t * 128:(dout_t + 1) * 128],
                                ones[:1, :], start=False, stop=True)
                            qk_f = p1.tile([128, 512], F32, tag="qk_f")
                            nc.scalar.copy(out=qk_f[:], in_=pj[:])
                            qk_p = p1.tile([128, 2, 512], F32R, tag="qk_p")
                            tmp = p1.tile([128, 512], F32, tag="qk_tmp")
                            nc.vector.tensor_copy(qk_p[:, 0, :], qk_f[:])
                            nc.vector.tensor_tensor(
                                out=tmp[:], in0=qk_f[:],
                                in1=qk_p[:, 0, :].bitcast(F32),
                                op=AT.subtract)
                            nc.vector.tensor_copy(qk_p[:, 1, :], tmp[:])
                            nc.sync.dma_start(
                                qkp_d[ch * 4:(ch + 1) * 4, :, :, dout_t, :]
                                .rearrange("tb p two q -> p two tb q"),
                                qk_p[:].bitcast(F32).rearrange(
                                    "p two (tb q) -> p two tb q", tb=4))

            # ---------------- P2: m-slices ----------------
            if stop_after >= 2:
                with tc.tile_pool(name="mkp", bufs=2) as mkp, \
                     tc.tile_pool(name="p2a", bufs=1) as p2a, \
                     tc.tile_pool(name="p2b", bufs=2) as p2b:
                    for e in range(NQ):
                        mkr = mkp.tile([128, DT, MQ], F32R, tag="mkr")
                        mke = mkp.tile([128, DT, MQ], F32R, tag="mke")
                        ACH = min(256, MQ)
                        for mc in range(MQ // ACH):
                            m0 = e * MQ + mc * ACH
                            brows = p2a.tile([128, ACH // 128, D], F32,
                                             tag="brows")
                            nc.sync.dma_start(
                                brows[:],
                                bank_shard[m0:m0 + ACH, :].rearrange(
                                    "(mb p) d -> p mb d", p=128))
                            bT = p2a.tile([128, DT, ACH], F32, tag="bT")
                            for mb in range(ACH // 128):
                                for dint in range(DT):
                                    pt = ps.tile([128, 128], F32, tag="pt_t")
                                    nc.tensor.transpose(
                                        out=pt[:],
                                        in_=brows[:, mb,
                                                  dint * 128:(dint + 1) * 128],
                                        identity=ident[:])
                                    nc.scalar.copy(
                                        out=bT[:, dint,
                                               mb * 128:(mb + 1) * 128],
                                        in_=pt[:])
                            for dout_t in range(DT):
                                wslice = p2a.tile([128, DT, 128], F32,
                                                  tag="wslice")
                                nc.sync.dma_start(wslice[:],
                                                  wkt_d[dout_t, :, :, :])
                                pj = ps.tile([128, 512], F32, tag="pj")
                                for dint in range(DT):
                                    nc.tensor.matmul(
                                        pj[:, :ACH], wslice[:, dint, :],
                                        bT[:, dint, :],
                                        start=(dint == 0),
                                        stop=(dint == DT - 1))
                                mk_f = p2a.tile([128, ACH], F32, tag="mk_f")
                                nc.scalar.copy(out=mk_f[:], in_=pj[:, :ACH])
                                tmp2 = p2a.tile([128, ACH], F32, tag="mk_tmp")
                                sl = slice(mc * ACH, (mc + 1) * ACH)
                                nc.vector.tensor_copy(mkr[:, dout_t, sl],
                                                      mk_f[:])
                                nc.vector.tensor_tensor(
                                    out=tmp2[:], in0=mk_f[:],
                                    in1=mkr[:, dout_t, sl].bitcast(F32),
                                    op=AT.subtract)
                                nc.vector.tensor_copy(mke[:, dout_t, sl],
                                                      tmp2[:])
                        for t in range(QT):
                            qkp_t = p2b.tile([128, 2, DT, 128], F32R,
                                             tag="qkp_t")
                            nc.sync.dma_start(qkp_t[:].bitcast(F32),
                                              qkp_d[t, :, :, :, :])
                            qkr_t = qkp_t[:, 0]
                            qke_t = qkp_t[:, 1]
                            sim = p2b.tile([128, MQ], F32, tag="sim")
                            passes = (((qkr_t, mkr), (qkr_t, mke),
                                       (qke_t, mkr))[:sim_passes])
                            # qkr_t/qke_t are APs [128, DT, 128]
                            for mc in range(NCH):
                                pj = ps.tile([128, CH], F32, tag="pj")
                                nmm = DT * len(passes)
                                i = 0
                                for dint in range(DT):
                                    sl = slice(mc * CH, (mc + 1) * CH)
                                    for (lh, rh) in passes:
                                        nc.tensor.matmul(
                                            pj[:, :CH], lh[:, dint, :],
                                            rh[:, dint, sl],
                                            start=(i == 0),
                                            stop=(i == nmm - 1))
                                        i += 1
                                nc.scalar.copy(
                                    out=sim[:, mc * CH:(mc + 1) * CH],
                                    in_=pj[:, :CH])
                            bvp = p2b.tile([128, 2, NB, 8], F32, tag="bvp")
                            bp = p2b.tile([128, NB, 8], U16, tag="bp")
                            if skip_topk:
                                nc.vector.tensor_copy(bvp[:, 0], sim[:, :NB * 8])
                            else:
                                for b in range(NB):
                                    blk = sim[:, b * 128:(b + 1) * 128]
                                    nc.vector.max(out=bvp[:, 0, b, :], in_=blk)
                                    nc.vector.max_index(
                                        out=bp[:, b, :],
                                        in_max=bvp[:, 0, b, :],
                                        in_values=blk)
                            nc.vector.tensor_copy(bvp[:, 1], bp[:])
                            nc.sync.dma_start(
                                lvp_d[t, :, e, :, :],
                                bvp[:].rearrange("p two nb k -> p (two nb k)")
                                .rearrange("p (two k) -> p two k", two=2))

            # ---------------- P3: local merge + A2A prep ----------------
            if stop_after >= 3:
                with tc.tile_pool(name="p3", bufs=2) as p3:
                    for t in range(QT):
                        cvp = p3.tile([128, NQ, 2, CW], F32, tag="cvp3")
                        nc.sync.dma_start(cvp[:], lvp_d[t, :, :, :, :])
                        cig = p3.tile([128, NQ, CW], U16, tag="cig3")
                        nc.vector.tensor_copy(
                            cig[:], cvp[:, :, 1, :])
                        nc.vector.tensor_tensor(
                            out=cig[:], in0=cig[:],
                            in1=boff[:].rearrange("p e nb k -> p e (nb k)"),
                            op=AT.add)
                        work = p3.tile([128, W2], F32, tag="work3")
                        nc.vector.tensor_copy(
                            work[:].rearrange("p (e k) -> p e k", e=NQ),
                            cvp[:, :, 0, :])
                        flv = p3.tile([128, K], F32, tag="flv")
                        fpos = p3.tile([128, K], U16, tag="fpos")
                        _topk32(nc, work, flv, fpos)
                        fli = p3.tile([128, K], U16, tag="fli")
                        _map_pos(nc, p3, fpos,
                                cig[:].rearrange("p e k -> p (e k)"),
                                ranks32, fli[:], W2)
                        nc.sync.dma_start(av_in[t * 128:(t + 1) * 128, :],
                                          flv[:])
                        nc.sync.dma_start(ai_in[t * 128:(t + 1) * 128, :],
                                          fli[:])

            # ---------------- P4: AllToAll ----------------
            if stop_after >= 4:
                if single_core:
                    nc.gpsimd.dma_start(av_out[:, :], av_in[:, :])
                    nc.gpsimd.dma_start(ai_out[:, :], ai_in[:, :])
                else:
                    nc.gpsimd.collective_compute(
                        "AllToAll", AT.bypass,
                        replica_groups=[list(range(CORES))],
                        ins=[av_in.opt()], outs=[av_out.opt()])
                    nc.gpsimd.collective_compute(
                        "AllToAll", AT.bypass,
                        replica_groups=[list(range(CORES))],
                        ins=[ai_in.opt()], outs=[ai_out.opt()])

            # ---------------- P5: global merge + gather ----------------
            if stop_after >= 5:
                with tc.tile_pool(name="p5", bufs=2) as p5:
                    for st in range(QTS):
                        cw = p5.tile([128, CORES, K], F32, tag="cw5")
                        ci = p5.tile([128, CORES, K], U16, tag="ci5")
                        nc.sync.dma_start(
                            cw[:], av_out[:, :].rearrange(
                                "(s q) k -> s q k",
                                s=CORES)[:, st * 128:(st + 1) * 128, :]
                            .rearrange("s p k -> p s k"))
                        nc.sync.dma_start(
                            ci[:], ai_out[:, :].rearrange(
                                "(s q) k -> s q k",
                                s=CORES)[:, st * 128:(st + 1) * 128, :]
                            .rearrange("s p k -> p s k"))
                        cig = p5.tile([128, CORES, K], U16, tag="cig")
                        nc.vector.tensor_tensor(out=cig[:], in0=ci[:],
                                                in1=shoff[:], op=AT.add)
                        gv = p5.tile([128, K], F32, tag="gv")
                        gpos = p5.tile([128, K], U16, tag="gpos")
                        _topk32(nc, cw[:].rearrange("p s k -> p (s k)"),
                               gv, gpos)
                        gidx = p5.tile([128, K], U16, tag="gidx")
                        _map_pos(nc, p5, gpos,
                                cig[:].rearrange("p s k -> p (s k)"),
                                ranks32, gidx[:], W3)
                        nc.sync.dma_start(fidx[st * 128:(st + 1) * 128, :],
                                          gidx[:])
                        gidx32 = p5.tile([128, K], I32, tag="gidx32")
                        nc.vector.tensor_copy(gidx32[:], gidx[:])
                        if skip_gather:
                            continue
                        for j in range(K):
                            row = p5.tile([128, D], F32, tag="row")
                            nc.gpsimd.indirect_dma_start(
                                out=row[:], out_offset=None,
                                in_=bank_full[:, :],
                                in_offset=bass.IndirectOffsetOnAxis(
                                    ap=gidx32[:, j:j + 1], axis=0))
                            nc.sync.dma_start(
                                retr[st * 128:(st + 1) * 128, j, :], row[:])

    nc.compile()
    return nc




_GRAPH_CACHE = {}


def kernel(**inputs):
    from concourse import bass_utils
    query = np.ascontiguousarray(np.asarray(inputs["query"]), dtype=np.float32)
    Wk = np.ascontiguousarray(np.asarray(inputs["Wk"]), dtype=np.float32)
    bk = np.ascontiguousarray(np.asarray(inputs["bk"]), dtype=np.float32)
    bank = np.ascontiguousarray(np.asarray(inputs["memory_bank"]),
                                dtype=np.float32)
    usage0 = np.asarray(inputs["memory_usage"])
    k = int(np.asarray(inputs["k"]))
    B, D = query.shape
    M = bank.shape[0]
    CORES = 8
    assert k == 32 and B == 4096 and M == 65536 and D == 1024

    key = (B, M, D)
    if key not in _GRAPH_CACHE:
        _GRAPH_CACHE[key] = _build_graph(B, M, D, CORES=CORES, NQ=8, K=k)
    nc = _GRAPH_CACHE[key]

    MS = M // CORES
    in_maps = []
    for c in range(CORES):
        in_maps.append({
            "query": query,
            "Wk": Wk,
            "bk": bk.reshape(1, D),
            "bank_shard": np.ascontiguousarray(bank[c * MS:(c + 1) * MS]),
            "bank_full": bank,
        })
    res = bass_utils.run_bass_kernel_spmd(nc, in_maps,
                                          core_ids=list(range(CORES)))
    QS = B // CORES
    retr = np.concatenate(
        [np.asarray(r["retr"]).reshape(QS, k, D) for r in res.results], axis=0)
    idx = np.concatenate([np.asarray(r["fidx"]) for r in res.results],
                         axis=0).astype(np.int64)
    usage = np.array(usage0, dtype=np.float32, copy=True)
    np.add.at(usage, idx.ravel(), np.float32(0.1))
    return retr, usage
